# revision 4
# baseline (speedup 1.0000x reference)
"""Correlation module kernel for 8 TRN2 NeuronCores.

Reference computation (per batch element n, pure data-parallel over N):
    A_n = X_n @ U_n^T / sqrt(D)          # [L, O]
    W_n = sigmoid(A_n) - 0.5             # = 0.5 * tanh(A_n / 2)
    F_n = W_n @ U_n                      # [L, D]

Shapes: x [L=512, N=64, D=512] f32, upfold [O=512, N=64, D=512] f32.
Sharding: N axis across 8 cores (8 batch elements per core), no comms.

Device kernel (per core, per n):
    MM1:  psum_AT[o, l] = sum_d uT[d, o] * xT[d, l]      (bf16 in, f32 acc)
    ACT:  w[o, l] = tanh(psum_AT * 1/(2*sqrt(D)))        (-> bf16)
    MM2:  psum_F[l, d] = sum_o w[o, l] * (0.5*u)[o, d]   (bf16 in, f32 acc)
    DVE:  f[l, d] = psum_F                               (f32)
    DMA out to y[l, n, d].

Host pre-arranges per-core inputs as bf16 in the exact layouts the PE
needs (d-major for MM1 operands, o-major for MM2's moving operand), so
the device does zero transposes and minimum HBM traffic.
"""

import numpy as np

L, O, N, D = 512, 512, 64, 512
NCORES = 8
NLOC = N // NCORES  # 8 batch elements per core
P = 128  # SBUF partitions
DB = D // P  # 4 d-blocks
OB = O // P  # 4 o-blocks
LB = L // P  # 4 l-blocks

_cache = {}


def _build_program():
    import concourse.bass as bass
    import concourse.mybir as mybir
    import concourse.tile as tile
    from concourse import bacc

    BF16 = mybir.dt.bfloat16
    F32 = mybir.dt.float32
    Tanh = mybir.ActivationFunctionType.Tanh

    nc = bacc.Bacc("TRN2", target_bir_lowering=False, debug=False)
    xt_d = nc.declare_dram_parameter("xt", [NLOC, D, L], BF16, isOutput=False)
    ut_d = nc.declare_dram_parameter("ut", [NLOC, D, O], BF16, isOutput=False)
    un_d = nc.declare_dram_parameter("un", [NLOC, O, D], BF16, isOutput=False)
    y_d = nc.declare_dram_parameter("y", [L, NLOC, D], F32, isOutput=True)

    s2 = 1.0 / (2.0 * float(np.sqrt(D)))  # tanh half-argument scale

    with tile.TileContext(nc) as tc:
        with (
            tc.tile_pool(name="xt", bufs=NLOC) as xt_pool,
            tc.tile_pool(name="ut", bufs=NLOC) as ut_pool,
            tc.tile_pool(name="un", bufs=NLOC) as un_pool,
            tc.tile_pool(name="w", bufs=2) as w_pool,
            tc.tile_pool(name="fo", bufs=2) as f_pool,
            tc.tile_pool(name="psa", bufs=2, space="PSUM") as psa_pool,
            tc.tile_pool(name="psf", bufs=2, space="PSUM") as psf_pool,
        ):
            for n in range(NLOC):
                # -- loads (d-major layouts land as [p, blk, free]) --
                xt_t = xt_pool.tile([P, DB, L], BF16, tag="xt")
                nc.sync.dma_start(
                    xt_t[:], xt_d[n].rearrange("(b p) l -> p b l", p=P)
                )
                ut_t = ut_pool.tile([P, DB, O], BF16, tag="ut")
                nc.sync.dma_start(
                    ut_t[:], ut_d[n].rearrange("(b p) o -> p b o", p=P)
                )
                un_t = un_pool.tile([P, OB, D], BF16, tag="un")
                nc.sync.dma_start(
                    un_t[:], un_d[n].rearrange("(b p) d -> p b d", p=P)
                )

                # -- MM1 + sigmoid: w[o, l] = tanh(scale * sum_d uT xT) --
                w_t = w_pool.tile([P, OB, L], BF16, tag="w")
                for ob in range(OB):
                    ps_a = psa_pool.tile([P, L], F32, tag="psa")
                    for db in range(DB):
                        nc.tensor.matmul(
                            ps_a[:],
                            lhsT=ut_t[:, db, bass.ts(ob, P)],
                            rhs=xt_t[:, db, :],
                            start=(db == 0),
                            stop=(db == DB - 1),
                        )
                    nc.scalar.activation(w_t[:, ob, :], ps_a[:], Tanh, scale=s2)

                # -- MM2: f[l, d] = sum_o w[o, l] * un[o, d] --
                f_t = f_pool.tile([P, LB, D], F32, tag="f")
                for lb in range(LB):
                    ps_f = psf_pool.tile([P, D], F32, tag="psf")
                    for ob in range(OB):
                        nc.tensor.matmul(
                            ps_f[:],
                            lhsT=w_t[:, ob, bass.ts(lb, P)],
                            rhs=un_t[:, ob, :],
                            start=(ob == 0),
                            stop=(ob == OB - 1),
                        )
                    nc.vector.tensor_copy(f_t[:, lb, :], ps_f[:])

                # -- store: y[l, n, d] --
                nc.scalar.dma_start(
                    y_d[:, n, :].rearrange("(b p) d -> p b d", p=P), f_t[:]
                )
    nc.compile()
    return nc


def _prepare_in_maps(x, u):
    import ml_dtypes

    bf16 = ml_dtypes.bfloat16
    in_maps = []
    for c in range(NCORES):
        ns = slice(c * NLOC, (c + 1) * NLOC)
        xs = x[:, ns, :]  # [L, NLOC, D]
        us = u[:, ns, :]  # [O, NLOC, D]
        in_maps.append(
            {
                # X^T per n: [NLOC, D, L]
                "xt": np.ascontiguousarray(xs.transpose(1, 2, 0)).astype(bf16),
                # U^T per n: [NLOC, D, O]
                "ut": np.ascontiguousarray(us.transpose(1, 2, 0)).astype(bf16),
                # U natural per n, pre-scaled by 0.5 (folds sigmoid's -0.5
                # via sigmoid(a)-0.5 = 0.5*tanh(a/2)): [NLOC, O, D]
                "un": (0.5 * us.transpose(1, 0, 2)).astype(bf16),
            }
        )
    return in_maps


def _run(inputs, trace=False, **spmd_kwargs):
    from concourse.bass_utils import run_bass_kernel_spmd

    x = np.asarray(inputs["x"], dtype=np.float32)
    u = np.asarray(inputs["upfold"], dtype=np.float32)
    assert x.shape == (L, N, D) and u.shape == (O, N, D)

    if "nc" not in _cache:
        _cache["nc"] = _build_program()
    nc = _cache["nc"]

    in_maps = _prepare_in_maps(x, u)
    res = run_bass_kernel_spmd(
        nc, in_maps, core_ids=list(range(NCORES)), trace=trace, **spmd_kwargs
    )
    out = np.concatenate([r["y"] for r in res.results], axis=1)  # [L, N, D]
    return np.ascontiguousarray(out, dtype=np.float32), res


def kernel(**inputs) -> np.ndarray:
    out, _ = _run(inputs, trace=False)
    return out
